# revision 2
# baseline (speedup 1.0000x reference)
"""Ragged-segment attention kernel for Trainium2 (8 NeuronCores, SPMD).

Reference semantics (per segment s with length n = lengths[s]):
    u      = tanh(context[s] @ W.T + b)            # [L, D]
    scores = context[s] @ u.T  (keys >= n masked)  # [L, L]
    attn   = softmax(scores, axis=-1)
    out[s] = (attn @ context[s]) * (query < n)

Strategy:
  * Host packs only the VALID tokens of each segment into per-core "bins" of
    128 tokens (segment rows at aligned offsets inside a bin).  Sorting by
    length and dealing rank r -> core r%8 makes the multiset of (offset,
    slot-length) identical across all 8 cores, so a single SPMD program works;
    per-slot padding to the max length within the 8-core rank group costs ~1%.
  * Device, per bin: C^T via PE transposes (f32r); u^T = tanh(W C^T + b) via
    f32r matmuls batched over a 4-bin group (moving dim 512 keeps f32r at
    full speed); per segment: bin-dense scores (f32r) + rank-1 key-bias
    matmul for masking pads; softmax via DVE rowmax(negated) + ACT exp with
    accumulated row sums + DVE reciprocal + row scale; attn^T block placed
    at its bin offset with a regular fp32 matmul against the identity; one
    block-diagonal out matmul per bin (f32r, N=512).
  * Host scatters valid rows back into the zero-initialised full output.
"""
import os
import numpy as np

import concourse.bacc as bacc
import concourse.mybir as mybir
import concourse.tile as tile
from concourse.bass_utils import run_bass_kernel_spmd

F32 = mybir.dt.float32
F32R = mybir.dt.float32r

N_CORES = 8
D = 512
BIN = 128          # tokens per bin (= SBUF partitions)
GROUP = 4          # bins per u-projection group (moving dim GROUP*128)
NEG = -1.0e30

LAST_RESULTS = {}  # test introspection: exec_time_ns etc.


# --------------------------------------------------------------------------
# host-side planning
# --------------------------------------------------------------------------

def _alignment(L):
    if L <= 32:
        return 32
    if L <= 64:
        return 64
    return 128


def _plan(lengths):
    """Compute the shared slot/bin layout.

    Returns (slots, n_bins, seg_ids) where
      slots: list of (bin_idx, offset, L_slot) -- identical for every core
      seg_ids[c][j]: segment id handled by core c in slot j
    """
    S = len(lengths)
    n_slots = S // N_CORES
    order = np.argsort(-lengths, kind="stable")
    seg_ids = [[int(order[N_CORES * j + c]) for j in range(n_slots)]
               for c in range(N_CORES)]
    # round up to a multiple of 4: f32r matmuls require even free sizes,
    # and the pad keys are masked via kbias anyway
    slot_len = [min(128, -(-int(lengths[order[N_CORES * j]]) // 4) * 4)
                for j in range(n_slots)]

    # first-fit decreasing with alignment into 128-token bins
    bins = []  # list of list of (off, L)
    slots = []
    for j, L in enumerate(slot_len):
        a = _alignment(L)
        placed = None
        for bi, occ in enumerate(bins):
            for off in range(0, BIN - L + 1, a):
                if all(off + L <= o or off >= o + ln for o, ln in occ):
                    placed = (bi, off)
                    break
            if placed:
                break
        if not placed:
            bins.append([])
            placed = (len(bins) - 1, 0)
        bi, off = placed
        bins[bi].append((off, L))
        slots.append((bi, off, L))

    n_bins = len(bins)
    n_bins = ((n_bins + GROUP - 1) // GROUP) * GROUP
    return slots, n_bins, seg_ids


# --------------------------------------------------------------------------
# device program
# --------------------------------------------------------------------------

def _build(slots, n_bins):
    nc = bacc.Bacc("TRN2", target_bir_lowering=False)
    T = n_bins * BIN

    cpk = nc.dram_tensor("cpk", [T, D], F32R, kind="ExternalInput")
    wt = nc.dram_tensor("wt", [128, 4 * D], F32R, kind="ExternalInput")
    bvec = nc.dram_tensor("bvec", [128, 4], F32, kind="ExternalInput")
    kbias = nc.dram_tensor("kbias", [1, T], F32R, kind="ExternalInput")
    opk = nc.dram_tensor("opk", [T, D], F32, kind="ExternalOutput")

    ident = nc.inline_tensor(np.eye(128, dtype=np.float32), name="ident")
    ones = nc.inline_tensor(np.ones((1, 128), dtype=np.float32), name="ones")

    # slots grouped by bin
    by_bin = [[] for _ in range(n_bins)]
    for bi, off, L in slots:
        by_bin[bi].append((off, L))

    with tile.TileContext(nc) as tc:
        with (
            tc.tile_pool(name="const", bufs=1) as cpool,
            tc.tile_pool(name="cb", bufs=2 * GROUP + 2) as cbp,
            tc.tile_pool(name="grp", bufs=2) as grp,
            tc.tile_pool(name="seg", bufs=4) as segp,
            tc.tile_pool(name="stat", bufs=6) as statp,
            tc.tile_pool(name="outp", bufs=3) as outp,
            tc.tile_pool(name="ups", bufs=2, space="PSUM") as ups,
            tc.tile_pool(name="scps", bufs=2, space="PSUM") as scps,
            tc.tile_pool(name="trps", bufs=2, space="PSUM") as trps,
            tc.tile_pool(name="ops", bufs=2, space="PSUM") as opsp,
        ):
            wt_sb = cpool.tile([128, 4, D], F32R, tag="wt")
            b_sb = cpool.tile([128, 4], F32, tag="b")
            id_r = cpool.tile([128, 128], F32R, tag="idr")
            id_f = cpool.tile([128, 128], F32, tag="idf")
            ones_r = cpool.tile([1, 128], F32R, tag="ones")
            nc.sync.dma_start(wt_sb[:], wt.ap().rearrange("p (c e) -> p c e", c=4))
            nc.sync.dma_start(b_sb[:], bvec[:])
            nc.sync.dma_start(id_r[:], ident.ap().bitcast(F32R))
            nc.sync.dma_start(id_f[:], ident[:])
            nc.sync.dma_start(ones_r[:], ones.ap().bitcast(F32R))

            for g in range(n_bins // GROUP):
                gbins = range(g * GROUP, (g + 1) * GROUP)

                # load bins, build C^T group tile [d_part, dchunk, bin, tok]
                ct = grp.tile([128, 4, GROUP, 128], F32R, tag="ct")
                kb_sb = grp.tile([1, GROUP * 128], F32R, tag="kb")
                nc.sync.dma_start(
                    kb_sb[:], kbias[:, g * GROUP * 128:(g + 1) * GROUP * 128])
                cbs = []
                for i, b in enumerate(gbins):
                    cbin = cbp.tile([128, D], F32R, tag="cbin")
                    cbs.append(cbin)
                    nc.sync.dma_start(cbin[:], cpk[b * BIN:(b + 1) * BIN, :])
                    for k in range(4):
                        pt = trps.tile([128, 128], F32R, tag="tr")
                        nc.tensor.transpose(
                            pt[:], cbin[:, k * 128:(k + 1) * 128], id_r[:])
                        nc.vector.tensor_copy(ct[:, k, i, :], pt[:])

                # u^T = tanh(W C^T + b) for the whole group
                ut = grp.tile([128, 4, GROUP, 128], F32R, tag="ut")
                for c in range(4):
                    ups_t = ups.tile([128, GROUP * 128], F32, tag="ups")
                    for k in range(4):
                        nc.tensor.matmul(
                            ups_t[:], wt_sb[:, k, c * 128:(c + 1) * 128],
                            ct[:, k, :, :], start=(k == 0), stop=(k == 3))
                    nc.scalar.activation(
                        ut[:, c, :, :], ups_t[:],
                        mybir.ActivationFunctionType.Tanh, bias=b_sb[:, c:c + 1])

                # per-bin attention
                for i, b in enumerate(gbins):
                    segs = by_bin[b]
                    if not segs:
                        continue
                    attn = segp.tile([128, 128], F32R, tag="attn")
                    nc.vector.memset(attn[:].bitcast(F32), 0.0)

                    for off, L in segs:
                        tb = i * 128 + off
                        sc = scps.tile([128, 128], F32, tag="sc")
                        for k in range(4):
                            nc.tensor.matmul(
                                sc[:L, :L],
                                ct[:, k, i, off:off + L],
                                ut[:, k, i, off:off + L],
                                start=(k == 0), stop=False)
                        nc.tensor.matmul(
                            sc[:L, :L], ones_r[:, :L],
                            kb_sb[:, tb:tb + L], start=False, stop=True)

                        nmax = statp.tile([128, 1], F32, tag="nmax")
                        sums = statp.tile([128, 1], F32, tag="sums")
                        recip = statp.tile([128, 1], F32, tag="recip")
                        expt = segp.tile([128, 128], F32, tag="expt")
                        nc.vector.tensor_reduce(
                            nmax[:L], sc[:L, :L], axis=mybir.AxisListType.X,
                            op=mybir.AluOpType.max, negate=True)
                        nc.scalar.activation(
                            expt[:L, :L], sc[:L, :L],
                            mybir.ActivationFunctionType.Exp,
                            bias=nmax[:L], accum_out=sums[:L])
                        nc.vector.reciprocal(recip[:L], sums[:L])
                        nc.vector.tensor_scalar_mul(
                            expt[:L, :L], expt[:L, :L], recip[:L])

                        # attn^T block at (off, off): fp32 matmul exp.T @ I
                        tp = trps.tile([128, 128], F32, tag="tr")
                        nc.tensor.matmul(
                            tp[off:off + L, off:off + L], expt[:L, :L],
                            id_f[:L, :L], start=True, stop=True,
                            tile_position=(0, off))
                        nc.vector.tensor_copy(
                            attn[off:off + L, off:off + L],
                            tp[off:off + L, off:off + L])

                    ops_t = opsp.tile([128, D], F32, tag="ops")
                    nc.tensor.matmul(ops_t[:], attn[:], cbs[i][:],
                                     start=True, stop=True)
                    osb = outp.tile([128, D], F32, tag="osb")
                    nc.vector.tensor_copy(osb[:], ops_t[:])
                    nc.sync.dma_start(opk[b * BIN:(b + 1) * BIN, :], osb[:])

    nc.compile()
    return nc


# --------------------------------------------------------------------------
# entry point
# --------------------------------------------------------------------------

_CACHE = {}


def kernel(context, lengths, W, b):
    context = np.asarray(context, dtype=np.float32)
    lengths = np.asarray(lengths, dtype=np.int32)
    W = np.asarray(W, dtype=np.float32)
    b = np.asarray(b, dtype=np.float32)
    S, Lmax, Din = context.shape
    assert Din == D and Lmax == 128 and S % N_CORES == 0

    slots, n_bins, seg_ids = _plan(lengths)
    T = n_bins * BIN

    key = (tuple(slots), n_bins)
    if key in _CACHE:
        nc = _CACHE[key]
    else:
        nc = _build(slots, n_bins)
        _CACHE[key] = nc

    wt = np.ascontiguousarray(
        W.T.reshape(4, 128, D).transpose(1, 0, 2).reshape(128, 4 * D))
    bvec = np.ascontiguousarray(b.reshape(4, 128).T)

    in_maps = []
    for c in range(N_CORES):
        cpk = np.zeros((T, D), np.float32)
        kb = np.full((1, T), NEG, np.float32)
        for j, (bi, off, _L) in enumerate(slots):
            s = seg_ids[c][j]
            n = int(lengths[s])
            r0 = bi * BIN + off
            cpk[r0:r0 + n] = context[s, :n]
            kb[0, r0:r0 + n] = 0.0
        in_maps.append({"cpk": cpk, "wt": wt, "bvec": bvec, "kbias": kb})

    trace = bool(int(os.environ.get("ATTN_TRACE", "0")))
    res = run_bass_kernel_spmd(nc, in_maps, list(range(N_CORES)), trace=trace)
    LAST_RESULTS["exec_time_ns"] = res.exec_time_ns
    LAST_RESULTS["mean_exec_time_ns"] = res.mean_exec_time_ns

    out = np.zeros((S, Lmax, D), np.float32)
    for c in range(N_CORES):
        opk = res.results[c]["opk"]
        for j, (bi, off, _L) in enumerate(slots):
            s = seg_ids[c][j]
            n = int(lengths[s])
            r0 = bi * BIN + off
            out[s, :n] = opk[r0:r0 + n]
    return out
